# revision 1
# baseline (speedup 1.0000x reference)
"""CARAFE (content-aware upsampling) Trainium2 Bass kernel.

Problem: features [2,64,64,128] f32, masks [2,128,128,25] f32 ->
out [2,128,128,128] f32; kernel_size=5, 2x nearest upsample, per-pixel
softmax over the 25-tap window, weighted sum of the 5x5 low-res patch.

Formulation: for each 8x16 output-pixel tile the 25 taps of all 128
pixels live inside an 8x12 low-res feature region (96 pixels). The
whole tile is then ONE matmul on the tensor engine:

    out[pix, c] = sum_p expW[p, pix] * Freg[p, c] / denom[pix]

where expW is the exp of the raw mask logits scattered (host-side, pure
data movement) into the [96 region, 128 pix] layout with -1e4 fill
(exp -> 0), and denom comes for free as a fused ones-column in the rhs.
exp runs on the scalar engine, reciprocal+scale on the vector engine.

Sharding: 8 cores = batch (2) x 4 row-bands of 32 output rows. Each
core gets a 20-row padded low-res feature band (4-row halo) and its 32
pre-scattered weight tiles.
"""

import os
import numpy as np
from contextlib import ExitStack

import concourse.bacc as bacc
import concourse.bass as bass
import concourse.tile as tile
import concourse.mybir as mybir
from concourse import bass_utils

B, H, W, MC = 2, 128, 128, 25
LH, LW, C = 64, 64, 128
K5 = 5
TILE_U, TILE_V = 8, 16     # output tile: 8 rows x 16 cols = 128 pixels
REG_R, REG_S = 8, 12       # low-res feature region covering one tile
NT_I, NT_J = 4, 8          # tiles per core: 32 rows/8 x 128 cols/16
N_TILES = NT_I * NT_J      # 32
N_CORES = 8
BAND = 32                  # output rows per core
NEG = np.float32(-1e4)     # exp(NEG) == 0 in fp32

_last_exec_time_ns = None
_cache = {}


def _build_program():
    nc = bacc.Bacc("TRN2", target_bir_lowering=False, debug=False)
    f32 = mybir.dt.float32
    fpad = nc.dram_tensor("fpad", [20, 68, C], f32, kind="ExternalInput")
    wt = nc.dram_tensor("wt", [N_TILES, REG_R * REG_S, 128], f32,
                        kind="ExternalInput")
    out = nc.dram_tensor("out", [BAND, W, C], f32, kind="ExternalOutput")

    with tile.TileContext(nc) as tc, ExitStack() as ctx:
        wt_pool = ctx.enter_context(tc.tile_pool(name="wt", bufs=3))
        ew_pool = ctx.enter_context(tc.tile_pool(name="ew", bufs=3))
        fr_pool = ctx.enter_context(tc.tile_pool(name="fr", bufs=3))
        ps_pool = ctx.enter_context(
            tc.tile_pool(name="ps", bufs=4, space=bass.MemorySpace.PSUM))
        sv_pool = ctx.enter_context(tc.tile_pool(name="sv", bufs=4))
        ot_pool = ctx.enter_context(tc.tile_pool(name="ot", bufs=3))

        for ti in range(NT_I):
            for tj in range(NT_J):
                t = ti * NT_J + tj
                lw = wt_pool.tile([96, 128], f32)
                nc.sync.dma_start(lw[:], wt[t])
                ew = ew_pool.tile([96, 128], f32)
                nc.scalar.activation(ew[:], lw[:],
                                     mybir.ActivationFunctionType.Exp)

                fr = fr_pool.tile([96, 132], f32)
                nc.sync.dma_start(
                    fr[:, 0:128],
                    fpad[4 * ti:4 * ti + REG_R, 8 * tj:8 * tj + REG_S, :])
                nc.gpsimd.memset(fr[:, 128:129], 1.0)

                ps = ps_pool.tile([128, 129], f32)
                nc.tensor.matmul(ps[:], ew[:], fr[:, 0:129])

                sinv = sv_pool.tile([128, 1], f32)
                nc.vector.reciprocal(sinv[:], ps[:, 128:129])
                ot = ot_pool.tile([128, C], f32)
                nc.vector.tensor_scalar_mul(ot[:], ps[:, 0:128], sinv[:])
                nc.sync.dma_start(
                    out[8 * ti:8 * ti + TILE_U, 16 * tj:16 * tj + TILE_V, :],
                    ot[:])

    nc.compile()
    return nc


def _scatter_indices():
    """Static (p, x) -> mask-channel map for one 8x16 tile.

    p = rr*12+ss indexes the 8x12 feature region, x = u*16+v the output
    pixel. Tap (di,dj) of pixel (u,v) reads region pixel
    (u//2+di, v//2+dj), so channel k = 5*di+dj lands at that p.
    """
    p = np.arange(REG_R * REG_S)
    rr, ss = p // REG_S, p % REG_S
    x = np.arange(TILE_U * TILE_V)
    u, v = x // TILE_V, x % TILE_V
    di = rr[:, None] - (u[None, :] // 2)
    dj = ss[:, None] - (v[None, :] // 2)
    valid = (di >= 0) & (di < K5) & (dj >= 0) & (dj < K5)
    kidx = np.where(valid, di * K5 + dj, 0)
    return valid, kidx, np.broadcast_to(x, (REG_R * REG_S, TILE_U * TILE_V))


def _prep_inputs(features, masks):
    features = np.ascontiguousarray(features, dtype=np.float32)
    masks = np.ascontiguousarray(masks, dtype=np.float32)

    fpad = np.zeros((B, LH + 4, LW + 4, C), np.float32)
    fpad[:, 2:2 + LH, 2:2 + LW] = features

    valid, kidx, xgrid = _scatter_indices()
    # masks -> (b, TI, u, TJ, v, k) -> (b, TI, TJ, x, k)
    mt = masks.reshape(B, H // TILE_U, TILE_U, NT_J, TILE_V, MC)
    mt = mt.transpose(0, 1, 3, 2, 4, 5).reshape(
        B, H // TILE_U, NT_J, TILE_U * TILE_V, MC)
    wt_all = mt[:, :, :, xgrid, kidx]          # [B, 16, 8, 96, 128]
    wt_all = np.where(valid, wt_all, NEG).astype(np.float32)

    in_maps = []
    for core in range(N_CORES):
        b, band = divmod(core, N_CORES // B)
        in_maps.append({
            "fpad": np.ascontiguousarray(fpad[b, 16 * band:16 * band + 20]),
            "wt": np.ascontiguousarray(
                wt_all[b, 4 * band:4 * band + 4].reshape(N_TILES, 96, 128)),
        })
    return in_maps


def kernel(features, masks):
    global _last_exec_time_ns
    if "nc" not in _cache:
        _cache["nc"] = _build_program()
    nc = _cache["nc"]

    in_maps = _prep_inputs(features, masks)
    trace = bool(os.environ.get("CARAFE_TRACE"))
    try:
        res = bass_utils.run_bass_kernel_spmd(
            nc, in_maps, core_ids=list(range(N_CORES)), trace=trace)
    except Exception:
        if not trace:
            raise
        res = bass_utils.run_bass_kernel_spmd(
            nc, in_maps, core_ids=list(range(N_CORES)), trace=False)
    _last_exec_time_ns = res.exec_time_ns

    out = np.empty((B, H, W, C), np.float32)
    for core in range(N_CORES):
        b, band = divmod(core, N_CORES // B)
        out[b, BAND * band:BAND * band + BAND] = res.results[core]["out"]
    return out


# revision 4
# speedup vs baseline: 2.2616x; 2.2616x over previous
"""CARAFE (content-aware upsampling) Trainium2 Bass kernel.

Problem: features [2,64,64,128] f32, masks [2,128,128,25] f32 ->
out [2,128,128,128] f32; kernel_size=5, 2x nearest upsample, per-pixel
softmax over the 25-tap window, weighted sum of the 5x5 low-res patch.

Formulation: for each 8x16 output-pixel tile the 25 taps of all 128
pixels live inside an 8x12 low-res feature region (96 pixels). The
whole tile is then ONE matmul on the tensor engine:

    out[pix, c] = sum_p expW[p, pix] * Freg[p, c] / denom[pix]

where expW is the exp of the raw mask logits scattered (host-side, pure
data movement) into the [96 region, 128 pix] layout with -1e4 fill
(exp -> 0), and denom comes for free as a fused ones-column in the rhs
(baked into the region layout host-side). exp runs on the scalar
engine, reciprocal+scale on the vector engine.

All DRAM traffic is host-prearranged to be fully contiguous: per core
only 12 DMAs (4x 384KB weight loads, 4x 396KB region loads, 4x 1MB
output stores).

Sharding: 8 cores = batch (2) x 4 row-bands of 32 output rows.
"""

import os
import numpy as np
from contextlib import ExitStack

import concourse.bacc as bacc
import concourse.bass as bass
import concourse.tile as tile
import concourse.mybir as mybir
from concourse import bass_utils

B, H, W, MC = 2, 128, 128, 25
LH, LW, C = 64, 64, 128
K5 = 5
TILE_U, TILE_V = 8, 16     # output tile: 8 rows x 16 cols = 128 pixels
REG_R, REG_S = 8, 12       # low-res feature region covering one tile
REG_P = REG_R * REG_S      # 96
NT_I, NT_J = 4, 8          # tiles per core: 32 rows/8 x 128 cols/16
N_CORES = 8
BAND = 32                  # output rows per core
RC = C + 1                 # region free width: 128 channels + ones col
NEG = np.float32(-1e4)     # exp(NEG) == 0 in fp32

_last_exec_time_ns = None
_cache = {}


def _build_program():
    nc = bacc.Bacc("TRN2", target_bir_lowering=False, debug=False)
    f32 = mybir.dt.float32
    # weight tiles, host-scattered:   [ti, region_pix, tj*128 pixels]
    wt = nc.dram_tensor("wt", [NT_I, REG_P, NT_J * 128], f32,
                        kind="ExternalInput")
    # feature regions + ones column:  [ti, region_pix, tj, 129]
    freg = nc.dram_tensor("freg", [NT_I, REG_P, NT_J * RC], f32,
                          kind="ExternalInput")
    # output, tile-major; host un-permutes: [ti, u, v, tj, c]
    out = nc.dram_tensor("out", [NT_I, TILE_U, TILE_V, NT_J, C], f32,
                         kind="ExternalOutput")

    with tile.TileContext(nc) as tc, ExitStack() as ctx:
        wt_pool = ctx.enter_context(tc.tile_pool(name="wt", bufs=2))
        ew_pool = ctx.enter_context(tc.tile_pool(name="ew", bufs=2))
        fr_pool = ctx.enter_context(tc.tile_pool(name="fr", bufs=2))
        ps_pool = ctx.enter_context(
            tc.tile_pool(name="ps", bufs=8, space=bass.MemorySpace.PSUM))
        sv_pool = ctx.enter_context(tc.tile_pool(name="sv", bufs=8))
        st_pool = ctx.enter_context(tc.tile_pool(name="st", bufs=2))

        for ti in range(NT_I):
            lwb = wt_pool.tile([REG_P, NT_J * 128], f32)
            nc.sync.dma_start(lwb[:], wt[ti])
            ewb = ew_pool.tile([REG_P, NT_J * 128], f32)
            nc.scalar.activation(ewb[:], lwb[:],
                                 mybir.ActivationFunctionType.Exp)

            frb = fr_pool.tile([REG_P, NT_J, RC], f32)
            nc.scalar.dma_start(frb[:], freg[ti])

            stage = st_pool.tile([128, NT_J, C], f32)
            for tj in range(NT_J):
                ps = ps_pool.tile([128, RC], f32)
                nc.tensor.matmul(ps[:], ewb[:, 128 * tj:128 * tj + 128],
                                 frb[:, tj, :])
                sinv = sv_pool.tile([128, 1], f32)
                nc.vector.reciprocal(sinv[:], ps[:, C:RC])
                nc.vector.tensor_scalar_mul(stage[:, tj, :], ps[:, 0:C],
                                            sinv[:])

            nc.sync.dma_start(out[ti], stage[:])

    nc.compile()
    return nc


def _scatter_indices():
    """Static (p, x) -> mask-channel map for one 8x16 tile.

    p = rr*12+ss indexes the 8x12 feature region, x = u*16+v the output
    pixel. Tap (di,dj) of pixel (u,v) reads region pixel
    (u//2+di, v//2+dj), so channel k = 5*di+dj lands at that p.
    """
    p = np.arange(REG_P)
    rr, ss = p // REG_S, p % REG_S
    x = np.arange(TILE_U * TILE_V)
    u, v = x // TILE_V, x % TILE_V
    di = rr[:, None] - (u[None, :] // 2)
    dj = ss[:, None] - (v[None, :] // 2)
    valid = (di >= 0) & (di < K5) & (dj >= 0) & (dj < K5)
    kidx = np.where(valid, di * K5 + dj, 0)
    return valid, kidx, np.broadcast_to(x, (REG_P, TILE_U * TILE_V))


def _prep_inputs(features, masks):
    features = np.ascontiguousarray(features, dtype=np.float32)
    masks = np.ascontiguousarray(masks, dtype=np.float32)

    # --- weights: scatter mask logits into the per-tile [96, 128] layout
    valid, kidx, xgrid = _scatter_indices()
    # masks -> (b, TI, u, TJ, v, k) -> (b, TI, TJ, x, k)
    mt = masks.reshape(B, H // TILE_U, TILE_U, NT_J, TILE_V, MC)
    mt = mt.transpose(0, 1, 3, 2, 4, 5).reshape(
        B, H // TILE_U, NT_J, TILE_U * TILE_V, MC)
    wt_all = mt[:, :, :, xgrid, kidx]          # [B, 16, TJ, 96, 128]
    wt_all = np.where(valid, wt_all, NEG).astype(np.float32)
    # -> [B, 16, 96, TJ, 128] so each ti band is one contiguous chunk
    wt_all = np.ascontiguousarray(wt_all.transpose(0, 1, 3, 2, 4))

    # --- feature regions (zero-padded borders) + ones column
    fpad = np.zeros((B, LH + 4, LW + 4, C), np.float32)
    fpad[:, 2:2 + LH, 2:2 + LW] = features
    p = np.arange(REG_P)
    ti_g = np.arange(H // TILE_U)
    tj_g = np.arange(NT_J)
    ridx = 4 * ti_g[:, None, None] + (p // REG_S)[None, :, None]  # [16,96,1]
    sidx = 8 * tj_g[None, None, :] + (p % REG_S)[None, :, None]   # [1,96,8]
    freg_all = fpad[:, ridx, sidx]             # [B, 16, 96, 8, 128]
    freg_all = np.concatenate(
        [freg_all,
         np.ones(freg_all.shape[:-1] + (1,), np.float32)], axis=-1)

    in_maps = []
    for core in range(N_CORES):
        b, band = divmod(core, N_CORES // B)
        in_maps.append({
            "wt": wt_all[b, 4 * band:4 * band + 4].reshape(
                NT_I, REG_P, NT_J * 128),
            "freg": np.ascontiguousarray(
                freg_all[b, 4 * band:4 * band + 4]).reshape(
                    NT_I, REG_P, NT_J * RC),
        })
    return in_maps


def kernel(features, masks):
    global _last_exec_time_ns
    if "nc" not in _cache:
        _cache["nc"] = _build_program()
    nc = _cache["nc"]

    in_maps = _prep_inputs(features, masks)
    trace = bool(os.environ.get("CARAFE_TRACE"))
    try:
        res = bass_utils.run_bass_kernel_spmd(
            nc, in_maps, core_ids=list(range(N_CORES)), trace=trace)
    except Exception:
        if not trace:
            raise
        res = bass_utils.run_bass_kernel_spmd(
            nc, in_maps, core_ids=list(range(N_CORES)), trace=False)
    _last_exec_time_ns = res.exec_time_ns

    out = np.empty((B, H, W, C), np.float32)
    for core in range(N_CORES):
        b, band = divmod(core, N_CORES // B)
        o = res.results[core]["out"]           # [ti, u, v, tj, c]
        o = o.transpose(0, 1, 3, 2, 4).reshape(BAND, W, C)  # rows, cols, c
        out[b, BAND * band:BAND * band + BAND] = o
    return out
